# revision 22
# baseline (speedup 1.0000x reference)
"""BroadcastAttention Trainium2 kernel (8 NeuronCores, data-parallel over batch).

Math per sample (C=512, N=4096, H=8 heads, HD=64):
    qkv = Wqkv @ x            # [H*(1+2HD), N]
    q[h,n], k[h,d,n], v[h,d,n] split per head
    s = softmax(q over n)     # [H, N]
    ctx[h,d] = sum_n k[h,d,n]*s[h,n]
    out = Wp @ (relu(v)*ctx) + bp

Key algebraic restructuring vs the straightforward formulation: the dense
K projection (a full [512,512]@[512,4096] matmul per sample, one third of
the FLOPs) is never computed. Since ctx_h = Wk_h @ (x @ s_h), we compute
t[c,h] = sum_n x[c,n]*exp(q[h,n]) and apply Wk to the tiny [C,H] result.

Scheduling (from perfetto trace analysis):
    - Matmul operands are bf16 (fp32 streams through the PE at half
      rate); PSUM accumulation stays fp32. Steady 512-col matmuls issue
      at ~216ns (the streaming floor).
    - x fp32->bf16 casts alternate ScalarE/VectorE. GpSimd casts are ~5x
      slower - not used.
    - x is transposed to xT via XBAR (dma_start_transpose). The ucode
      descriptor-gen occupies the issuing engine ~1.25us per [128,1024]
      block, so transposes are issued PER PIECE as soon as that piece is
      cast, split across the two HWDGE queues (Sync + Scalar). Bunching
      them at sample end serializes ~20us on Sync and stalls the P phase
      (the old design's main bottleneck).
    - t accumulates per chunk (lag 1) via M=8 matmuls at distinct 32-col
      tile_position groups into one pre-zeroed PSUM bank; only the last
      chunk's group plus the small z/ctx chain remains at sample end,
      hidden under the last V chunk.
    - ctx is folded into the P-phase weights (wps = wp * ctx per
      contraction channel), so the V phase never waits on ctx.
    - The next sample's x casts + transposes are interleaved into the
      current sample's P phase (32 chunk slots fit 16 casts + 16
      transposes exactly); its HBM loads issue on the idle GpSimd SWDGE
      queue so they never queue behind transpose ucode on Sync.
    - y is stored bf16 (host upcasts); halves store traffic and the
      kernel tail. Total rel err ~5e-3 vs the 2e-2 gate.
"""

import sys

for _p in ("/opt/trn_rl_repo",):
    if _p not in sys.path:
        sys.path.insert(0, _p)

from contextlib import ExitStack

import ml_dtypes
import numpy as np

import concourse.bass as bass
import concourse.mybir as mybir
import concourse.tile as tile
from concourse import bacc
from concourse.bass_utils import run_bass_kernel_spmd

# Problem constants (hardcoded per contract; kernel.py must be self-contained).
B, C, N = 16, 512, 4096
H, HD = 8, 64
NCORES = 8
BPC = B // NCORES  # samples per core
CT = C // 128      # 4 contraction/partition tiles of 128
NT = N // 128      # 32 n-tiles
FREE = 512         # matmul moving free-dim chunk
NCH = N // FREE    # 8 chunks
FP = mybir.dt.float32
BF = mybir.dt.bfloat16

# Results of the last run (for test harness introspection).
LAST_RESULTS = None


def _build(has_qkv_bias: bool, has_p_bias: bool) -> bass.Bass:
    nc = bacc.Bacc("TRN2", target_bir_lowering=False, debug=False)

    x_d = nc.declare_dram_parameter("x", [BPC, C, N], FP, isOutput=False)
    wq_d = nc.declare_dram_parameter("wqT", [C, H], BF, isOutput=False)
    wv_d = nc.declare_dram_parameter("wvT", [C, C], BF, isOutput=False)
    wk_d = nc.declare_dram_parameter("wkO", [C, C], BF, isOutput=False)
    wp_d = nc.declare_dram_parameter("wpT", [C, C], BF, isOutput=False)
    eselB_d = nc.declare_dram_parameter("eselB", [128, C], BF, isOutput=False)
    eselT_d = nc.declare_dram_parameter("eselT", [H, 128], FP, isOutput=False)
    bq_d = nc.declare_dram_parameter("bq", [1, H], BF, isOutput=False)
    bkc_d = nc.declare_dram_parameter("bkcol", [C], FP, isOutput=False)
    bv_d = nc.declare_dram_parameter("bv", [1, C], BF, isOutput=False)
    bp_d = nc.declare_dram_parameter("bp", [C], FP, isOutput=False)
    y_d = nc.declare_dram_parameter("y", [BPC, C, N], BF, isOutput=True)

    AF = mybir.ActivationFunctionType
    OP = mybir.AluOpType

    with tile.TileContext(nc) as tc, ExitStack() as ctx:
        consts = ctx.enter_context(tc.tile_pool(name="consts", bufs=1))
        xstage = ctx.enter_context(tc.tile_pool(name="xstage", bufs=14))
        xpool = ctx.enter_context(tc.tile_pool(name="xpool", bufs=1))
        xtpool = ctx.enter_context(tc.tile_pool(name="xtpool", bufs=1))
        apool = ctx.enter_context(tc.tile_pool(name="apool", bufs=1))
        spool = ctx.enter_context(tc.tile_pool(name="spool", bufs=2))
        wpspool = ctx.enter_context(tc.tile_pool(name="wpspool", bufs=2))
        opool = ctx.enter_context(tc.tile_pool(name="opool", bufs=6))
        small = ctx.enter_context(tc.tile_pool(name="small", bufs=2))
        ps_tr = ctx.enter_context(tc.tile_pool(name="ps_tr", bufs=2, space="PSUM"))
        ps_q = ctx.enter_context(tc.tile_pool(name="ps_q", bufs=2, space="PSUM"))
        ps_ctx = ctx.enter_context(tc.tile_pool(name="ps_ctx", bufs=1, space="PSUM"))
        ps_mm = ctx.enter_context(tc.tile_pool(name="ps_mm", bufs=3, space="PSUM"))

        # ---- constants / weights into SBUF ----
        # GpSimd (SWDGE) queue: startup weights, away from x on Sync.
        wq_sb = consts.tile([128, CT, H], BF)
        wv_sb = consts.tile([128, CT, C], BF)
        wk_sb = consts.tile([128, CT, C], BF)
        wp_sb = consts.tile([128, CT, C], BF)
        eselB_sb = consts.tile([128, C], BF)
        eselT_sb = consts.tile([H, 128], FP)
        ones_col = consts.tile([128, 1], FP)

        # All weights load upfront on GpSimd (SWDGE) — the slower SWDGE
        # transfer rate (~half of HWDGE) is fine for weights, and this
        # keeps the Sync HWDGE queue free for x loads + transposes.
        nc.gpsimd.memset(ones_col[:], 1.0)
        for wct in range(CT):
            wsl = slice(wct * 128, (wct + 1) * 128)
            nc.gpsimd.dma_start(out=wq_sb[:, wct, :], in_=wq_d[wsl, :])
            nc.gpsimd.dma_start(out=wv_sb[:, wct, :], in_=wv_d[wsl, :])
        nc.gpsimd.dma_start(out=eselB_sb[:], in_=eselB_d[:, :])
        nc.gpsimd.dma_start(out=eselT_sb[:], in_=eselT_d[:, :])
        for wct in range(CT):
            wsl = slice(wct * 128, (wct + 1) * 128)
            nc.gpsimd.dma_start(out=wk_sb[:, wct, :], in_=wk_d[wsl, :])
            nc.gpsimd.dma_start(out=wp_sb[:, wct, :], in_=wp_d[wsl, :])
        if has_qkv_bias:
            bq_sb = consts.tile([1, H], BF)
            bkc_sb = consts.tile([128, CT], FP)
            bv_sb = consts.tile([1, C], BF)
            ones_row = consts.tile([1, FREE], BF)
            nc.gpsimd.dma_start(out=bq_sb[:], in_=bq_d[:, :])
            nc.gpsimd.dma_start(
                out=bkc_sb[:], in_=bkc_d.rearrange("(o p) -> p o", p=128)
            )
            nc.gpsimd.dma_start(out=bv_sb[:], in_=bv_d[:, :])
            nc.gpsimd.memset(ones_row[:], 1.0)
        if has_p_bias:
            bp_sb = consts.tile([128, CT], FP)
            nc.gpsimd.dma_start(
                out=bp_sb[:], in_=bp_d.rearrange("(o p) -> p o", p=128)
            )

        # x is staged in [128, 1024] tiles keyed (b, row, qcol): row is a
        # 128-channel block, qcol a quarter of the n axis. Each
        # dma_start_transpose has a ~5us fixed cost regardless of size,
        # so transposes stay FULL-ROW (4 per sample, all on Sync); the
        # load/cast order is arranged so whole rows complete early and
        # the 4 transposes pipeline during the chunk loop (b==0) or the
        # previous P phase (b>0) instead of bunching at sample end.
        staged = {}
        QW = 1024  # quarter width

        def emit_ld(b, r, p, queue):
            xst = xstage.tile([128, QW], FP, tag="xst", name="xst")
            queue.dma_start(
                out=xst[:],
                in_=x_d[b, r * 128:(r + 1) * 128, p * QW:(p + 1) * QW],
            )
            staged[(b, r, p)] = xst

        cast_rr = [0]

        def emit_cast(b, x_sb, r, p):
            xst = staged.pop((b, r, p))
            eng = (nc.scalar, nc.vector)[cast_rr[0] % 2]
            cast_rr[0] += 1
            dst = x_sb[:, r, p * QW:(p + 1) * QW]
            if eng is nc.scalar:
                nc.scalar.copy(dst, xst[:])
            else:
                eng.tensor_copy(out=dst, in_=xst[:])

        def emit_T(x_sb, xT_sb, r, queue=None):
            # XBAR transpose of one full x row-block (identical op to the
            # proven full-sample design, just issued earlier).
            (queue or nc.sync).dma_start_transpose(
                out=xT_sb[:, :, r * 128:(r + 1) * 128],
                in_=x_sb[:, r, :],
            )

        def emit_t_group(chk, scoresT, xT_sb, ctx_big):
            # 4 M=8 matmuls at distinct 32-col tile_position groups run
            # concurrently; accumulate into the pre-zeroed ctx_big bank.
            for j in range(4):
                nt = chk * 4 + j
                nc.tensor.matmul(
                    ctx_big[32 * j:32 * j + H, :],
                    scoresT[:, nt, :], xT_sb[:, nt, :],
                    start=False, stop=(chk == NCH - 1),
                    skip_group_check=True,
                    tile_position=(0, 32 * j),
                )

        def alloc_sample():
            xT_sb = xtpool.tile([128, NT, C], BF, tag="xT", name="xT_sb")
            scoresT = spool.tile([128, NT, H], BF, tag="scoresT", name="scoresT")
            a_sb = apool.tile([128, CT, N], BF, tag="a_sb", name="a_sb")
            wps_sb = wpspool.tile([128, CT, C], BF, tag="wps", name="wps_sb")
            ctx_big = ps_ctx.tile([128, C], FP, tag="ctx", name="ctx_big")
            nc.vector.memset(ctx_big[:], 0.0)
            return xT_sb, scoresT, a_sb, wps_sb, ctx_big

        x_sb = xpool.tile([128, CT, N], BF, tag="x_sb", name="x_sb")
        cur = alloc_sample()

        for b in range(BPC):
            xT_sb, scoresT, a_sb, wps_sb, ctx_big = cur

            if b == 0:
                # ALL of batch 0's x loads issue upfront on Sync (HWDGE
                # — SWDGE transfers at only ~half the rate) in strict
                # need-order: heads (cols 0-2048) column-major for
                # compute start, then tails row-major so whole rows
                # complete early and the full-row transposes (emitted at
                # the cast boundaries below, AFTER all loads in the Sync
                # queue) pipeline during chunks 2-7.
                for p in (0, 1):
                    for r in range(CT):
                        emit_ld(b, r, p, nc.sync)
                for r in range(CT):
                    for p in (2, 3):
                        emit_ld(b, r, p, nc.sync)
                # boundary -> list of (kind, args) staging actions.
                boundary = {
                    0: [("c", r, 0) for r in range(CT)],
                    2: [("c", r, 1) for r in range(CT)],
                    3: [("c", 0, 2), ("c", 0, 3), ("c", 1, 2), ("c", 1, 3),
                        ("T", 0), ("T", 1)],
                    4: [("c", 2, 2), ("c", 2, 3), ("c", 3, 2), ("c", 3, 3),
                        ("T", 2), ("T", 3)],
                }
            else:
                boundary = {}

            for chk in range(NCH):
                for act in boundary.get(chk, ()):
                    if act[0] == "c":
                        emit_cast(b, x_sb, act[1], act[2])
                    else:
                        # All transposes MUST issue from the Sync queue:
                        # dma_start_transpose from the Activation queue
                        # produces corrupt data (measured: every sample
                        # whose odd rows transposed via Scalar came out
                        # ~50% wrong; all-Sync samples were exact).
                        emit_T(x_sb, xT_sb, act[1])

                # ---- q pass for the 4 n-tiles of this chunk ----
                for j4 in range(4):
                    nt = chk * 4 + j4
                    nsl = slice(nt * 128, (nt + 1) * 128)
                    q_ps = ps_q.tile([128, H], FP, tag="q8", name="q_ps")
                    for ct in range(CT):
                        xsl = x_sb[:, ct, nsl]
                        last = (ct == CT - 1) and not has_qkv_bias
                        nc.tensor.matmul(
                            q_ps[:], xsl, wq_sb[:, ct, :],
                            start=(ct == 0), stop=last,
                        )
                    if has_qkv_bias:
                        nc.tensor.matmul(
                            q_ps[:], ones_row[:, 0:128], bq_sb[:],
                            start=False, stop=True,
                        )
                    nc.scalar.activation(
                        out=scoresT[:, nt, :], in_=q_ps[:], func=AF.Exp
                    )

                def emit_V(i_list, scalar_evict=False):
                    csl = slice(chk * FREE, (chk + 1) * FREE)
                    for i in i_list:
                        v_ps = ps_mm.tile(
                            [128, FREE], FP, tag="mm512", name="v_ps"
                        )
                        for ct in range(CT):
                            last = (ct == CT - 1) and not has_qkv_bias
                            nc.tensor.matmul(
                                v_ps[:],
                                wv_sb[:, ct, i * 128:(i + 1) * 128],
                                x_sb[:, ct, csl],
                                start=(ct == 0), stop=last,
                            )
                        if has_qkv_bias:
                            nc.tensor.matmul(
                                v_ps[:], bv_sb[:, i * 128:(i + 1) * 128],
                                ones_row[:], start=False, stop=True,
                            )
                        if i % 2 == 0 and not scalar_evict:
                            nc.vector.tensor_scalar_max(
                                out=a_sb[:, i, csl], in0=v_ps[:], scalar1=0.0
                            )
                        else:
                            nc.scalar.activation(
                                out=a_sb[:, i, csl], in_=v_ps[:], func=AF.Relu
                            )

                if chk < NCH - 1:
                    # ---- V phase for this chunk (no ctx dependency) ----
                    emit_V(range(CT))
                    continue

                # ---- last chunk: V out-tiles 0-2 run FIRST so the PE
                # chews them while the last transpose lands (t-groups
                # need all of xT); then the 8 t-groups contiguously
                # (other matmul chains interleaved between the open
                # tile_position accumulation groups corrupt ctx); then
                # the z/ctx chain with V(3) slotted between its tiny PE
                # ops to cover the DVE round trips. V7's evictions all
                # go to Scalar so Vector runs the chain unobstructed. ----
                zpart = small.tile([128, H], FP, tag="zpart", name="zpart")
                nc.vector.reduce_sum(
                    out=zpart[:],
                    in_=scoresT[:].rearrange("p nt h -> p h nt"),
                    axis=mybir.AxisListType.X,
                )
                emit_V([0, 1, 2], scalar_evict=True)
                for g in range(NCH):
                    emit_t_group(g, scoresT, xT_sb, ctx_big)
                z_ps = ps_q.tile([H, 1], FP, tag="q8", name="z_ps")
                nc.tensor.matmul(
                    z_ps[:], zpart[:], ones_col[:], start=True, stop=True
                )
                invz = small.tile([H, 1], FP, tag="invz", name="invz")
                nc.vector.reciprocal(out=invz[:], in_=z_ps[:])
                emit_V([3], scalar_evict=True)
                zrow_ps = ps_q.tile([128, 1], FP, tag="q8", name="zrow_ps")
                nc.tensor.matmul(
                    zrow_ps[:], eselT_sb[:], invz[:], start=True, stop=True
                )
                zrow = small.tile([128, 1], FP, tag="zrow", name="zrow")
                nc.vector.tensor_copy(out=zrow[:], in_=zrow_ps[:])
                ctxcopy = small.tile([128, C], BF, tag="ctxcopy", name="ctxcopy")
                nc.vector.tensor_scalar_mul(
                    out=ctxcopy[:], in0=ctx_big[:], scalar1=zrow[:]
                )
                # ctx[ch] = sum_c Wk[ch,c] * t[c,h(ch)]: broadcast t to
                # all channels of its head (eselB matmul, which also
                # combines the 4 substreams), then a fused elementwise
                # multiply-reduce against Wk.
                # ctxv held as 4 separate [128,1] tiles so the wps fold
                # can scale with full-tile scalar operands on either
                # engine; the fold is pipelined right behind each
                # reduce (it is the last gate before the P phase).
                for i in range(CT):
                    tb_ps = ps_tr.tile([128, C], FP, tag="xt", name="tb_ps")
                    nc.tensor.matmul(
                        tb_ps[:], eselB_sb[:, i * 128:(i + 1) * 128],
                        ctxcopy[:], start=True, stop=True,
                    )
                    junk = small.tile([128, C], BF, tag="junk", name="junk")
                    nc.vector.tensor_tensor(
                        out=junk[:], in0=tb_ps[:], in1=wk_sb[:, i, :],
                        op=OP.mult,
                    )
                    ctxv_i = small.tile([128, 1], FP, tag="ctxv", name="ctxv")
                    nc.vector.reduce_sum(
                        out=ctxv_i[:], in_=junk[:],
                        axis=mybir.AxisListType.X,
                    )
                    if has_qkv_bias:
                        ctxvb_i = small.tile(
                            [128, 1], FP, tag="ctxv", name="ctxvb"
                        )
                        nc.vector.tensor_tensor(
                            out=ctxvb_i[:], in0=ctxv_i[:],
                            in1=bkc_sb[:, i:i + 1], op=OP.add,
                        )
                        ctxv_i = ctxvb_i
                    if i % 2 == 0:
                        nc.scalar.activation(
                            out=wps_sb[:, i, :], in_=wp_sb[:, i, :],
                            func=AF.Identity, scale=ctxv_i[:],
                        )
                    else:
                        nc.vector.tensor_scalar_mul(
                            out=wps_sb[:, i, :], in0=wp_sb[:, i, :],
                            scalar1=ctxv_i[:],
                        )

            # ---- prefetch next sample: HBM loads row-major on the idle
            # GpSimd SWDGE queue; casts + full-row transposes are
            # interleaved into the P-phase chunk slots below (32 slots
            # hold 16 casts + 4 transposes), so the next chunk loop
            # starts with x and xT fully resident. ----
            nxt = b + 1 if b + 1 < BPC else None
            interleave = []
            if nxt is not None:
                for r in range(CT):
                    for p in range(4):
                        emit_ld(nxt, r, p, nc.sync)
                x_sb = xpool.tile([128, CT, N], BF, tag="x_sb", name="x_sb")
                cur = alloc_sample()
                nxT = cur[0]
                for r in range(CT):
                    for p in range(4):
                        interleave.append((emit_cast, (nxt, x_sb, r, p)))
                    interleave.append((emit_T, (x_sb, nxT, r)))
            ivi = [0]

            def drain_interleave(k=1):
                for _ in range(k):
                    if ivi[0] < len(interleave):
                        fn, args = interleave[ivi[0]]
                        fn(*args)
                        ivi[0] += 1

            # ---- P phase: output projection (ctx-folded weights) ----
            HSTG = N // 2
            for o in range(CT):
                for half in range(2):
                    o_sb = opool.tile([128, HSTG], BF, tag="osb", name="o_sb")
                    for hc in range(NCH // 2):
                        chk = half * (NCH // 2) + hc
                        p_ps = ps_mm.tile([128, FREE], FP, tag="mm512", name="p_ps")
                        csl = slice(chk * FREE, (chk + 1) * FREE)
                        for c2 in range(CT):
                            nc.tensor.matmul(
                                p_ps[:],
                                wps_sb[:, c2, o * 128:(o + 1) * 128],
                                a_sb[:, c2, csl],
                                start=(c2 == 0), stop=(c2 == CT - 1),
                            )
                        osl = slice(hc * FREE, (hc + 1) * FREE)
                        # Alternate evictions DVE/ScalarE to split load.
                        if has_p_bias:
                            if chk % 2 == 0:
                                nc.vector.tensor_scalar_add(
                                    o_sb[:, osl], in0=p_ps[:],
                                    scalar1=bp_sb[:, o:o + 1],
                                )
                            else:
                                nc.scalar.add(
                                    o_sb[:, osl], p_ps[:], add=bp_sb[:, o:o + 1]
                                )
                        else:
                            if chk % 2 == 0:
                                nc.vector.tensor_copy(o_sb[:, osl], p_ps[:])
                            else:
                                nc.scalar.copy(o_sb[:, osl], p_ps[:])
                        drain_interleave(1)
                    ysl = y_d[b, o * 128:(o + 1) * 128,
                              half * HSTG:(half + 1) * HSTG]
                    if b == BPC - 1 and o == CT - 1:
                        # Final tile: store per chunk so the last DMA is
                        # small — shortens the kernel tail.
                        for qs in range(HSTG // FREE):
                            nc.gpsimd.dma_start(
                                out=ysl[:, qs * FREE:(qs + 1) * FREE],
                                in_=o_sb[:, qs * FREE:(qs + 1) * FREE],
                            )
                    else:
                        nc.gpsimd.dma_start(out=ysl, in_=o_sb[:])
            # anything left (b==BPC-2 with fewer slots than ops)
            drain_interleave(len(interleave))

    nc.compile()
    return nc


_NC_CACHE = {}


def kernel(x, Wqkv, bqkv, Wp, bp):
    global LAST_RESULTS
    x = np.ascontiguousarray(np.asarray(x, dtype=np.float32))
    Wqkv = np.asarray(Wqkv, dtype=np.float32)
    bqkv = np.asarray(bqkv, dtype=np.float32)
    Wp = np.asarray(Wp, dtype=np.float32)
    bp = np.asarray(bp, dtype=np.float32)

    # Host-side weight layout prep (tiny, one-time).
    bf16 = ml_dtypes.bfloat16
    r = Wqkv.reshape(H, 1 + 2 * HD, C)
    wqT = np.ascontiguousarray(r[:, 0, :].T).astype(bf16)              # [C, H]
    wvT = np.ascontiguousarray(r[:, 1 + HD:, :].reshape(C, C).T).astype(bf16)
    wkO = np.ascontiguousarray(r[:, 1:1 + HD, :].reshape(C, C)).astype(bf16)
    wpT = np.ascontiguousarray(Wp.T).astype(bf16)                      # [C, o]
    rb = bqkv.reshape(H, 1 + 2 * HD)
    bq = np.ascontiguousarray(rb[:, 0].reshape(1, H)).astype(bf16)
    bkcol = np.ascontiguousarray(rb[:, 1:1 + HD].reshape(C)).astype(np.float32)
    bv = np.ascontiguousarray(rb[:, 1 + HD:].reshape(1, C)).astype(bf16)
    ch = np.arange(C)
    p128 = np.arange(128)
    eselB = ((p128[:, None] % 32) == (ch[None, :] // HD)).astype(bf16)
    eselT = ((np.arange(H)[:, None]) == (p128[None, :] % 32)).astype(np.float32)

    has_qkv_bias = bool(np.any(bqkv != 0.0))
    has_p_bias = bool(np.any(bp != 0.0))

    key = (has_qkv_bias, has_p_bias)
    if key not in _NC_CACHE:
        _NC_CACHE[key] = _build(*key)
    nc = _NC_CACHE[key]

    shared = {
        "wqT": wqT, "wvT": wvT, "wkO": wkO, "wpT": wpT,
        "eselB": eselB, "eselT": eselT,
        "bq": bq, "bkcol": bkcol, "bv": bv, "bp": bp,
    }
    in_maps = [
        {"x": x[i * BPC:(i + 1) * BPC], **shared} for i in range(NCORES)
    ]
    LAST_RESULTS = run_bass_kernel_spmd(nc, in_maps, list(range(NCORES)))
    out = np.concatenate(
        [LAST_RESULTS.results[i]["y"] for i in range(NCORES)], axis=0
    )
    return out.astype(np.float32)


if __name__ == "__main__":
    rng = np.random.default_rng(0)
    x = rng.standard_normal((B, C, N), dtype=np.float32)
    Wqkv = (rng.standard_normal((H * (1 + 2 * HD), C), dtype=np.float32) * 0.02)
    bqkv = np.zeros((H * (1 + 2 * HD),), np.float32)
    Wp = rng.standard_normal((C, C), dtype=np.float32) * 0.02
    bp = np.zeros((C,), np.float32)
    y = kernel(x, Wqkv, bqkv, Wp, bp)
    print("out", y.shape, y.dtype)
